# revision 2
# baseline (speedup 1.0000x reference)
"""Trainium2 Bass kernel v3 for nn_DMHA_3255585210402 (retrieval_knn DMHA).

v2 post-mortem (409us vs 331us baseline) found three regressions, all fixed
here:
- dma_start issue costs ~650ns SERIAL time on the issuing sequencer
  (DIRECT2D).  v2's 4-way split loads/stores added ~50us of issue time,
  starving phase A (underruns reset the PE clock ramp: ~3us of half-speed
  after every gap) and serializing a 30us store tail.  v3 minimizes
  dma_start count, orders issues in exact consumption order, and spreads
  them over both hwdge queues (sync + scalar).
- The DVE mask-preload put the vector engine into the score->exp->attn
  chain; head tails stalled ~1-2us each.  v3 masks on the PE itself: a
  [128,128] identity-matmul writes the causal triangle (0/-1e30) into the
  scores psum (start of the accumulation group), the narrowed score matmul
  accumulates onto it, so masking costs 55ns of PE stream and no
  cross-engine dependency.  Columns left of the triangle are handled by
  four persistent pre-zeroed diagonal wt tiles (exp writes only [off:],
  zeros never change).
- Everything else kept from v2: all-bf16 operands (0.5% error vs 2e-2
  budget), softmax denominators via bf16 DVE add-tree + one ones-matmul
  per (chunk, head), lookahead-3 software pipelining, x resident in SBUF.
"""

import math

import numpy as np
import ml_dtypes

import concourse.bass as bass
import concourse.mybir as mybir
import concourse.tile as tile
from concourse import bacc
from concourse.bass_utils import run_bass_kernel_spmd

B, T, D = 2, 2048, 2048
H, HD = 16, 128
G = 4              # head-groups (cores per batch)
GH = H // G        # heads per core
GF = GH * HD       # projected features per core (512)
NCORES = 8
P = 128            # partitions
TQ = 512           # tq chunk width (psum bank)
F32 = mybir.dt.float32
BF16 = mybir.dt.bfloat16

DK = D // P        # 16 contraction chunks for projections
NTQ = T // TQ      # 4 query chunks
NTK = T // P       # 16 key chunks
GP = TQ // P       # diagonal tiles per query chunk (4)
NEG = -1.0e30


def _body(tc, xT, xg, wqT, wkT, woT, cT, bqT, bkT, ones, ident, mtri, out):
    nc = tc.nc
    rsqrt_hd = 1.0 / math.sqrt(HD)
    mult = mybir.AluOpType.mult

    with (
        tc.tile_pool(name="const", bufs=1) as const,
        tc.tile_pool(name="resB", bufs=1) as resB,
    ):
        qT_sb = resB.tile([P, GH, T], BF16)
        kT_sb = resB.tile([P, GH, T], BF16)
        xg_sb = resB.tile([P, NTK, GF], BF16)
        wo_sb = resB.tile([P, GH, D], BF16)
        oT_sb = resB.tile([P, GH, T], BF16)
        # persistent diagonal wt tiles; [:, :g*P] stays zero forever
        dwt = [resB.tile([P, TQ], BF16, name=f"dwt{g}") for g in range(GP)]

        ones_sb = const.tile([P, 1], BF16)
        ident_sb = const.tile([P, P], BF16)
        mtri_sb = const.tile([P, P], BF16)
        bq_sb = const.tile([HD, GH], F32)
        bk_sb = const.tile([HD, GH], F32)
        cT_sb = const.tile([HD, GH], F32)

        for g in range(GP):
            nc.vector.memset(dwt[g], 0.0)

        # --- phase A: q/k projections, transposed layout ---
        with (
            tc.tile_pool(name="wqk", bufs=1) as wqk,
            tc.tile_pool(name="xa", bufs=1) as xa,
            tc.tile_pool(name="psA", bufs=8, space="PSUM") as psA,
        ):
            wq_sb = wqk.tile([P, DK, GF], BF16)
            wk_sb = wqk.tile([P, DK, GF], BF16)
            xT_sb = xa.tile([P, DK, T], BF16)
            wqr = wqT.rearrange("(n p) f -> p n f", p=P)
            wkr = wkT.rearrange("(n p) f -> p n f", p=P)

            # scalar-queue dge: the 16 first-chunk x tiles, issued before
            # any activation is enqueued
            for dk in range(DK):
                nc.scalar.dma_start(
                    out=xT_sb[:, dk, 0:TQ], in_=xT[dk * P : (dk + 1) * P, 0:TQ]
                )
            # sync-queue dge: weights in consumption order (dk-major),
            # first chunk split 2-way for latency
            for half in range(2):
                sl = slice(half * 64, (half + 1) * 64)
                nc.sync.dma_start(out=wq_sb[sl, 0, :], in_=wqr[sl, 0, :])
            for half in range(2):
                sl = slice(half * 64, (half + 1) * 64)
                nc.sync.dma_start(out=wk_sb[sl, 0, :], in_=wkr[sl, 0, :])
            for dk in range(1, DK):
                nc.sync.dma_start(out=wq_sb[:, dk, :], in_=wqr[:, dk, :])
                nc.sync.dma_start(out=wk_sb[:, dk, :], in_=wkr[:, dk, :])

            from concourse import library_config
            with tc.high_priority():
                nc.gpsimd.load_library(library_config.attn)

            nc.sync.dma_start(out=ones_sb, in_=ones)
            nc.sync.dma_start(out=ident_sb, in_=ident)
            nc.sync.dma_start(out=mtri_sb, in_=mtri)
            nc.sync.dma_start(out=bq_sb, in_=bqT)
            nc.sync.dma_start(out=bk_sb, in_=bkT)
            nc.sync.dma_start(out=cT_sb, in_=cT)

            for tci in range(1, NTQ):
                tsl = slice(tci * TQ, (tci + 1) * TQ)
                for dk in range(DK):
                    nc.sync.dma_start(
                        out=xT_sb[:, dk, tsl], in_=xT[dk * P : (dk + 1) * P, tsl]
                    )
            for i in range(NTK):
                nc.sync.dma_start(
                    out=xg_sb[:, i, :], in_=xg[i * P : (i + 1) * P, :]
                )
            wor = woT.rearrange("(m p) d -> p m d", p=P)
            for m in range(GH):
                nc.sync.dma_start(out=wo_sb[:, m, :], in_=wor[:, m, :])

            for tci in range(NTQ):
                tsl = slice(tci * TQ, (tci + 1) * TQ)
                ps = [
                    psA.tile([P, TQ], F32, name="psA_t", tag="psA_t")
                    for _ in range(2 * GH)
                ]
                for dk in range(DK):
                    for w, w_sb in enumerate((wq_sb, wk_sb)):
                        for h in range(GH):
                            nc.tensor.matmul(
                                ps[w * GH + h],
                                w_sb[:, dk, h * HD : (h + 1) * HD],
                                xT_sb[:, dk, tsl],
                                start=(dk == 0),
                                stop=(dk == DK - 1),
                            )
                for w, dstT, bias in ((0, qT_sb, bq_sb), (1, kT_sb, bk_sb)):
                    for h in range(GH):
                        nc.scalar.activation(
                            dstT[:, h, tsl],
                            ps[w * GH + h],
                            mybir.ActivationFunctionType.Identity,
                            bias=bias[:, h : h + 1],
                        )

        # --- phases B+C: pipelined attention + output projection ---
        with (
            tc.tile_pool(name="wt", bufs=6) as wtp,
            tc.tile_pool(name="tr", bufs=10) as trp,
            tc.tile_pool(name="rb", bufs=2) as rbp,
            tc.tile_pool(name="small", bufs=2) as smp,
            tc.tile_pool(name="ct", bufs=4) as ctp,
            tc.tile_pool(name="psS", bufs=4, space="PSUM") as psS,
            tc.tile_pool(name="psO", bufs=3, space="PSUM") as psO,
            tc.tile_pool(name="psSum", bufs=1, space="PSUM") as psSum,
        ):
            LOOK = 3
            pending = [None]

            def emit_normalize():
                h, j, ps_o, rb = pending[0]
                nc.vector.scalar_tensor_tensor(
                    out=oT_sb[:, h, j * TQ : (j + 1) * TQ],
                    in0=ps_o,
                    scalar=cT_sb[:, h : h + 1],
                    in1=rb,
                    op0=mult,
                    op1=mult,
                )
                pending[0] = None

            def emit_B_chunk(j):
                nkk = (j + 1) * GP
                qsl = slice(j * TQ, (j + 1) * TQ)
                flat = [(h, i) for h in range(GH) for i in range(nkk)]
                n = len(flat)
                wt_of = {}
                ps_o_of = {}
                tree = {h: [None] * 6 for h in range(GH)}
                for t in range(n + LOOK):
                    if t >= LOOK:
                        h, i = flat[t - LOOK]
                        wt = wt_of.pop(t - LOOK)
                        if i == 0:
                            ps_o_of[h] = psO.tile([P, TQ], F32, name="ps_o")
                        nc.tensor.matmul(
                            ps_o_of[h],
                            xg_sb[:, i, h * HD : (h + 1) * HD],
                            wt,
                            start=(i == 0),
                            stop=(i == nkk - 1),
                        )
                        cur, lvl = wt, 0
                        tr = tree[h]
                        while tr[lvl] is not None:
                            nw = trp.tile([P, TQ], BF16, name="tr")
                            nc.vector.tensor_add(nw, tr[lvl], cur)
                            tr[lvl] = None
                            cur, lvl = nw, lvl + 1
                        tr[lvl] = cur
                        if i == nkk - 1:
                            rem = [x for x in tr if x is not None]
                            while len(rem) > 1:
                                nw = trp.tile([P, TQ], BF16, name="tr")
                                nc.vector.tensor_add(nw, rem[0], rem[1])
                                rem = [nw] + rem[2:]
                            ps_sum = psSum.tile([1, TQ], F32, name="ps_sum")
                            nc.tensor.matmul(
                                ps_sum, ones_sb, rem[0], start=True, stop=True
                            )
                            recip = smp.tile([1, TQ], F32, name="recip")
                            nc.vector.reciprocal_approx_fast(
                                out=recip, in_=ps_sum
                            )
                            rb = rbp.tile([P, TQ], F32, name="rb")
                            nc.gpsimd.partition_broadcast(rb, recip)
                            if pending[0] is not None:
                                emit_normalize()
                            pending[0] = (h, j, ps_o_of.pop(h), rb)

                    if t < n:
                        h, i = flat[t]
                        g = i - GP * j
                        ps_s = psS.tile([P, TQ], F32, name="ps_s", tag="ps_s")
                        if g >= 0:
                            off = g * P
                            # causal triangle into psum, then narrowed scores
                            nc.tensor.matmul(
                                ps_s[:, off : off + P],
                                ident_sb,
                                mtri_sb,
                                start=True,
                                stop=False,
                                skip_group_check=True,
                            )
                            nc.tensor.matmul(
                                ps_s[:, off : off + P],
                                kT_sb[:, h, i * P : (i + 1) * P],
                                qT_sb[:, h, j * TQ + off : j * TQ + off + P],
                                start=False,
                                stop=True,
                                skip_group_check=True,
                            )
                            if off + P < TQ:
                                nc.tensor.matmul(
                                    ps_s[:, off + P :],
                                    kT_sb[:, h, i * P : (i + 1) * P],
                                    qT_sb[:, h, j * TQ + off + P : (j + 1) * TQ],
                                    start=True,
                                    stop=True,
                                    skip_group_check=True,
                                )
                            wt = dwt[g]
                            nc.scalar.activation(
                                wt[:, off:], ps_s[:, off:],
                                mybir.ActivationFunctionType.Exp,
                                scale=rsqrt_hd,
                            )
                        else:
                            nc.tensor.matmul(
                                ps_s,
                                kT_sb[:, h, i * P : (i + 1) * P],
                                qT_sb[:, h, qsl],
                                start=True,
                                stop=True,
                            )
                            wt = wtp.tile([P, TQ], BF16, name="wt")
                            nc.scalar.activation(
                                wt, ps_s, mybir.ActivationFunctionType.Exp,
                                scale=rsqrt_hd,
                            )
                        wt_of[t] = wt
            def emit_C(j, final=False):
                qsl = slice(j * TQ, (j + 1) * TQ)
                for dk in range(DK):
                    ps = psS.tile([P, TQ], F32, name="psC_t", tag="ps_s")
                    for m in range(GH):
                        nc.tensor.matmul(
                            ps,
                            wo_sb[:, m, dk * P : (dk + 1) * P],
                            oT_sb[:, m, qsl],
                            start=(m == 0),
                            stop=(m == GH - 1),
                        )
                    ct = ctp.tile([P, TQ], BF16, name="ct")
                    nc.scalar.copy(ct, ps)
                    dst = out[dk * P : (dk + 1) * P, qsl]
                    if final and dk >= DK - 2:
                        nc.sync.dma_start(out=dst[0:64], in_=ct[0:64])
                        nc.sync.dma_start(out=dst[64:128], in_=ct[64:128])
                    else:
                        nc.sync.dma_start(out=dst, in_=ct)

            for j in range(NTQ):
                emit_B_chunk(j)
                if j > 0:
                    emit_C(j - 1)
            emit_normalize()
            emit_C(NTQ - 1, final=True)


def build_program():
    nc = bacc.Bacc(
        "TRN2", target_bir_lowering=False, debug=False, num_devices=NCORES
    )
    xT = nc.dram_tensor("xT", [D, T], BF16, kind="ExternalInput").ap()
    xg = nc.dram_tensor("xg", [T, GF], BF16, kind="ExternalInput").ap()
    wqT = nc.dram_tensor("wqT", [D, GF], BF16, kind="ExternalInput").ap()
    wkT = nc.dram_tensor("wkT", [D, GF], BF16, kind="ExternalInput").ap()
    woT = nc.dram_tensor("woT", [GF, D], BF16, kind="ExternalInput").ap()
    cT = nc.dram_tensor("cT", [HD, GH], F32, kind="ExternalInput").ap()
    bqT = nc.dram_tensor("bqT", [HD, GH], F32, kind="ExternalInput").ap()
    bkT = nc.dram_tensor("bkT", [HD, GH], F32, kind="ExternalInput").ap()
    ones = nc.dram_tensor("ones", [P, 1], BF16, kind="ExternalInput").ap()
    ident = nc.dram_tensor("ident", [P, P], BF16, kind="ExternalInput").ap()
    mtri = nc.dram_tensor("mtri", [P, P], BF16, kind="ExternalInput").ap()
    out = nc.dram_tensor("out", [D, T], BF16, kind="ExternalOutput").ap()

    with tile.TileContext(nc) as tc:
        _body(tc, xT, xg, wqT, wkT, woT, cT, bqT, bkT, ones, ident, mtri, out)
    nc.compile()
    return nc


_NC_CACHE = None
LAST_RESULT = None
TRACE = False


def kernel(x, Wq, bq, Wk, bk, Wvq, bvq, v_keys, v_embed, Wo, bo):
    global _NC_CACHE, LAST_RESULT
    bf = ml_dtypes.bfloat16
    x = np.asarray(x, np.float32)
    Wq = np.asarray(Wq, np.float32)
    bq = np.asarray(bq, np.float32)
    Wk = np.asarray(Wk, np.float32)
    bk = np.asarray(bk, np.float32)
    v_embed = np.asarray(v_embed, np.float32)
    Wo = np.asarray(Wo, np.float32)
    bo = np.asarray(bo, np.float32)

    c = 2.0 * v_embed[:G].sum(axis=0)
    p = np.arange(P)[:, None]
    f = np.arange(P)[None, :]
    mtri = np.where(f >= p, 0.0, NEG).astype(bf)
    in_maps = []
    for core in range(NCORES):
        b, g = divmod(core, G)
        gsl = slice(g * GF, (g + 1) * GF)
        in_maps.append(
            {
                "xT": np.ascontiguousarray(x[b].T.astype(bf)),
                "xg": np.ascontiguousarray(x[b][:, gsl].astype(bf)),
                "wqT": np.ascontiguousarray(Wq[gsl, :].T.astype(bf)),
                "wkT": np.ascontiguousarray(Wk[gsl, :].T.astype(bf)),
                "woT": np.ascontiguousarray(Wo[:, gsl].T.astype(bf)),
                "cT": np.ascontiguousarray(c[gsl].reshape(GH, HD).T),
                "bqT": np.ascontiguousarray(bq[gsl].reshape(GH, HD).T),
                "bkT": np.ascontiguousarray(bk[gsl].reshape(GH, HD).T),
                "ones": np.ones((P, 1), bf),
                "ident": np.eye(P, dtype=bf),
                "mtri": np.ascontiguousarray(mtri),
            }
        )

    if _NC_CACHE is None:
        _NC_CACHE = build_program()
    res = run_bass_kernel_spmd(
        _NC_CACHE, in_maps, list(range(NCORES)), trace=TRACE
    )
    LAST_RESULT = res

    out = np.zeros((B, T, D), np.float32)
    for core in range(NCORES):
        b = core // G
        out[b] += res.results[core]["out"].astype(np.float32).T
    out += bo[None, None, :]
    return out


if __name__ == "__main__":
    nc = build_program()
    print("built ok")


# revision 3
# speedup vs baseline: 1.0141x; 1.0141x over previous
"""Trainium2 Bass kernel v3 for nn_DMHA_3255585210402 (retrieval_knn DMHA).

v2 post-mortem (409us vs 331us baseline) found three regressions, all fixed
here:
- dma_start issue costs ~650ns SERIAL time on the issuing sequencer
  (DIRECT2D).  v2's 4-way split loads/stores added ~50us of issue time,
  starving phase A (underruns reset the PE clock ramp: ~3us of half-speed
  after every gap) and serializing a 30us store tail.  v3 minimizes
  dma_start count, orders issues in exact consumption order, and spreads
  them over both hwdge queues (sync + scalar).
- The DVE mask-preload put the vector engine into the score->exp->attn
  chain; head tails stalled ~1-2us each.  v3 masks on the PE itself: a
  [128,128] identity-matmul writes the causal triangle (0/-1e30) into the
  scores psum (start of the accumulation group), the narrowed score matmul
  accumulates onto it, so masking costs 55ns of PE stream and no
  cross-engine dependency.  Columns left of the triangle are handled by
  four persistent pre-zeroed diagonal wt tiles (exp writes only [off:],
  zeros never change).
- Everything else kept from v2: all-bf16 operands (0.5% error vs 2e-2
  budget), softmax denominators via bf16 DVE add-tree + one ones-matmul
  per (chunk, head), lookahead-3 software pipelining, x resident in SBUF.
"""

import math

import numpy as np
import ml_dtypes

import concourse.bass as bass
import concourse.mybir as mybir
import concourse.tile as tile
from concourse import bacc
from concourse.bass_utils import run_bass_kernel_spmd

B, T, D = 2, 2048, 2048
H, HD = 16, 128
G = 4              # head-groups (cores per batch)
GH = H // G        # heads per core
GF = GH * HD       # projected features per core (512)
NCORES = 8
P = 128            # partitions
TQ = 512           # tq chunk width (psum bank)
F32 = mybir.dt.float32
BF16 = mybir.dt.bfloat16

DK = D // P        # 16 contraction chunks for projections
NTQ = T // TQ      # 4 query chunks
NTK = T // P       # 16 key chunks
GP = TQ // P       # diagonal tiles per query chunk (4)
NEG = -1.0e30


def _body(tc, xT, xg, wqT, wkT, woT, cT, bqT, bkT, ones, ident, mtri, out):
    nc = tc.nc
    rsqrt_hd = 1.0 / math.sqrt(HD)
    mult = mybir.AluOpType.mult

    with (
        tc.tile_pool(name="const", bufs=1) as const,
        tc.tile_pool(name="resB", bufs=1) as resB,
    ):
        qT_sb = resB.tile([P, GH, T], BF16)
        kT_sb = resB.tile([P, GH, T], BF16)
        xg_sb = resB.tile([P, NTK, GF], BF16)
        wo_sb = resB.tile([P, GH, D], BF16)
        oT_sb = resB.tile([P, GH, T], BF16)
        # persistent diagonal wt tiles; [:, :g*P] stays zero forever
        dwt = [resB.tile([P, TQ], BF16, name=f"dwt{g}") for g in range(GP)]

        ones_sb = const.tile([P, 1], BF16)
        ident_sb = const.tile([P, P], BF16)
        mtri_sb = const.tile([P, P], BF16)
        bq_sb = const.tile([HD, GH], F32)
        bk_sb = const.tile([HD, GH], F32)
        cT_sb = const.tile([HD, GH], F32)

        for g in range(GP):
            nc.vector.memset(dwt[g], 0.0)

        # --- phase A: q/k projections, transposed layout ---
        with (
            tc.tile_pool(name="wqk", bufs=1) as wqk,
            tc.tile_pool(name="xa", bufs=1) as xa,
            tc.tile_pool(name="psA", bufs=8, space="PSUM") as psA,
        ):
            wq_sb = wqk.tile([P, DK, GF], BF16)
            wk_sb = wqk.tile([P, DK, GF], BF16)
            xT_sb = xa.tile([P, DK, T], BF16)
            wqr = wqT.rearrange("(n p) f -> p n f", p=P)
            wkr = wkT.rearrange("(n p) f -> p n f", p=P)

            # scalar-queue dge: the 16 first-chunk x tiles, issued before
            # any activation is enqueued
            for dk in range(DK):
                nc.scalar.dma_start(
                    out=xT_sb[:, dk, 0:TQ], in_=xT[dk * P : (dk + 1) * P, 0:TQ]
                )
            # sync-queue dge: weights in consumption order (dk-major),
            # first chunk split 2-way for latency
            for half in range(2):
                sl = slice(half * 64, (half + 1) * 64)
                nc.sync.dma_start(out=wq_sb[sl, 0, :], in_=wqr[sl, 0, :])
            for half in range(2):
                sl = slice(half * 64, (half + 1) * 64)
                nc.sync.dma_start(out=wk_sb[sl, 0, :], in_=wkr[sl, 0, :])
            for dk in range(1, DK):
                nc.sync.dma_start(out=wq_sb[:, dk, :], in_=wqr[:, dk, :])
                nc.sync.dma_start(out=wk_sb[:, dk, :], in_=wkr[:, dk, :])

            from concourse import library_config
            with tc.high_priority():
                nc.gpsimd.load_library(library_config.attn)

            nc.sync.dma_start(out=ones_sb, in_=ones)
            nc.sync.dma_start(out=ident_sb, in_=ident)
            nc.sync.dma_start(out=mtri_sb, in_=mtri)
            nc.sync.dma_start(out=bq_sb, in_=bqT)
            nc.sync.dma_start(out=bk_sb, in_=bkT)
            nc.sync.dma_start(out=cT_sb, in_=cT)

            for tci in range(1, NTQ):
                tsl = slice(tci * TQ, (tci + 1) * TQ)
                for dk in range(DK):
                    nc.sync.dma_start(
                        out=xT_sb[:, dk, tsl], in_=xT[dk * P : (dk + 1) * P, tsl]
                    )
            for i in range(NTK):
                nc.sync.dma_start(
                    out=xg_sb[:, i, :], in_=xg[i * P : (i + 1) * P, :]
                )
            wor = woT.rearrange("(m p) d -> p m d", p=P)
            for m in range(GH):
                nc.sync.dma_start(out=wo_sb[:, m, :], in_=wor[:, m, :])

            for tci in range(NTQ):
                tsl = slice(tci * TQ, (tci + 1) * TQ)
                ps = [
                    psA.tile([P, TQ], F32, name="psA_t", tag="psA_t")
                    for _ in range(2 * GH)
                ]
                for dk in range(DK):
                    for w, w_sb in enumerate((wq_sb, wk_sb)):
                        for h in range(GH):
                            nc.tensor.matmul(
                                ps[w * GH + h],
                                w_sb[:, dk, h * HD : (h + 1) * HD],
                                xT_sb[:, dk, tsl],
                                start=(dk == 0),
                                stop=(dk == DK - 1),
                            )
                for w, dstT, bias in ((0, qT_sb, bq_sb), (1, kT_sb, bk_sb)):
                    for h in range(GH):
                        nc.scalar.activation(
                            dstT[:, h, tsl],
                            ps[w * GH + h],
                            mybir.ActivationFunctionType.Identity,
                            bias=bias[:, h : h + 1],
                        )

        # --- phases B+C: pipelined attention + output projection ---
        with (
            tc.tile_pool(name="wt", bufs=6) as wtp,
            tc.tile_pool(name="tr", bufs=10) as trp,
            tc.tile_pool(name="rb", bufs=2) as rbp,
            tc.tile_pool(name="small", bufs=2) as smp,
            tc.tile_pool(name="ct", bufs=4) as ctp,
            tc.tile_pool(name="psS", bufs=4, space="PSUM") as psS,
            tc.tile_pool(name="psO", bufs=3, space="PSUM") as psO,
            tc.tile_pool(name="psSum", bufs=1, space="PSUM") as psSum,
        ):
            LOOK = 3
            pending = [None]

            def emit_normalize():
                h, j, ps_o, rb = pending[0]
                nc.vector.scalar_tensor_tensor(
                    out=oT_sb[:, h, j * TQ : (j + 1) * TQ],
                    in0=ps_o,
                    scalar=cT_sb[:, h : h + 1],
                    in1=rb,
                    op0=mult,
                    op1=mult,
                )
                pending[0] = None

            def B_steps(j):
                """One yield per pipeline step.  SS bundle carries the whole
                softmax-denominator chain (tree adds, ones-matmul, recip,
                broadcast, deferred normalize) since it depends only on the
                exp, not on the attention matmul; the AT bundle is a single
                matmul LOOK steps behind."""
                nkk = (j + 1) * GP
                qsl = slice(j * TQ, (j + 1) * TQ)
                flat = [(h, i) for h in range(GH) for i in range(nkk)]
                n = len(flat)
                wt_of = {}
                ps_o_of = {}
                tree = {h: [None] * 6 for h in range(GH)}
                for t in range(n + LOOK):
                    if t >= LOOK:
                        h, i = flat[t - LOOK]
                        wt = wt_of.pop(t - LOOK)
                        if i == 0:
                            ps_o_of[h] = psO.tile([P, TQ], F32, name="ps_o")
                        g = i - GP * j
                        if g >= 1:
                            off = g * P
                            nc.tensor.matmul(
                                ps_o_of[h][:, off:],
                                xg_sb[:, i, h * HD : (h + 1) * HD],
                                wt[:, off:],
                                start=False,
                                stop=(i == nkk - 1),
                                skip_group_check=True,
                            )
                        else:
                            nc.tensor.matmul(
                                ps_o_of[h],
                                xg_sb[:, i, h * HD : (h + 1) * HD],
                                wt,
                                start=(i == 0),
                                stop=(i == nkk - 1),
                                skip_group_check=True,
                            )
                    if t < n:
                        h, i = flat[t]
                        g = i - GP * j
                        ps_s = psS.tile([P, TQ], F32, name="ps_s", tag="ps_s")
                        if g >= 0:
                            off = g * P
                            nc.tensor.matmul(
                                ps_s[:, off : off + P],
                                ident_sb,
                                mtri_sb,
                                start=True,
                                stop=False,
                                skip_group_check=True,
                            )
                            nc.tensor.matmul(
                                ps_s[:, off : off + P],
                                kT_sb[:, h, i * P : (i + 1) * P],
                                qT_sb[:, h, j * TQ + off : j * TQ + off + P],
                                start=False,
                                stop=True,
                                skip_group_check=True,
                            )
                            if off + P < TQ:
                                nc.tensor.matmul(
                                    ps_s[:, off + P :],
                                    kT_sb[:, h, i * P : (i + 1) * P],
                                    qT_sb[:, h, j * TQ + off + P : (j + 1) * TQ],
                                    start=True,
                                    stop=True,
                                    skip_group_check=True,
                                )
                            wt = dwt[g]
                            nc.scalar.activation(
                                wt[:, off:], ps_s[:, off:],
                                mybir.ActivationFunctionType.Exp,
                                scale=rsqrt_hd,
                            )
                        else:
                            nc.tensor.matmul(
                                ps_s,
                                kT_sb[:, h, i * P : (i + 1) * P],
                                qT_sb[:, h, qsl],
                                start=True,
                                stop=True,
                            )
                            wt = wtp.tile([P, TQ], BF16, name="wt")
                            nc.scalar.activation(
                                wt, ps_s, mybir.ActivationFunctionType.Exp,
                                scale=rsqrt_hd,
                            )
                        wt_of[t] = wt
                        cur, lvl = wt, 0
                        tr = tree[h]
                        while tr[lvl] is not None:
                            nw = trp.tile([P, TQ], BF16, name="tr")
                            nc.vector.tensor_add(nw, tr[lvl], cur)
                            tr[lvl] = None
                            cur, lvl = nw, lvl + 1
                        tr[lvl] = cur
                        if i == nkk - 1:
                            rem = [x for x in tr if x is not None]
                            while len(rem) > 1:
                                nw = trp.tile([P, TQ], BF16, name="tr")
                                nc.vector.tensor_add(nw, rem[0], rem[1])
                                rem = [nw] + rem[2:]
                            ps_sum = psSum.tile([1, TQ], F32, name="ps_sum")
                            nc.tensor.matmul(
                                ps_sum, ones_sb, rem[0], start=True, stop=True
                            )
                            recip = smp.tile([1, TQ], F32, name="recip")
                            nc.vector.reciprocal_approx_fast(
                                out=recip, in_=ps_sum
                            )
                            rb = rbp.tile([P, TQ], F32, name="rb")
                            nc.gpsimd.partition_broadcast(rb, recip)
                            if pending[0] is not None:
                                emit_normalize()
                            pending[0] = (h, j, ps_o_of[h], rb)
                    yield

            def run_steps(gen, k=-1):
                while k != 0:
                    try:
                        next(gen)
                    except StopIteration:
                        return
                    k -= 1

            def emit_C(j, final=False):
                qsl = slice(j * TQ, (j + 1) * TQ)
                for dk in range(DK):
                    ps = psS.tile([P, TQ], F32, name="psC_t", tag="ps_s")
                    for m in range(GH):
                        nc.tensor.matmul(
                            ps,
                            wo_sb[:, m, dk * P : (dk + 1) * P],
                            oT_sb[:, m, qsl],
                            start=(m == 0),
                            stop=(m == GH - 1),
                        )
                    ct = ctp.tile([P, TQ], BF16, name="ct")
                    nc.scalar.copy(ct, ps)
                    dst = out[dk * P : (dk + 1) * P, qsl]
                    if final and dk >= DK - 2:
                        nc.sync.dma_start(out=dst[0:64], in_=ct[0:64])
                        nc.sync.dma_start(out=dst[64:128], in_=ct[64:128])
                    else:
                        nc.sync.dma_start(out=dst, in_=ct)

            gens = [B_steps(j) for j in range(NTQ)]
            run_steps(gens[0], LOOK)
            for j in range(NTQ):
                run_steps(gens[j])
                if j + 1 < NTQ:
                    run_steps(gens[j + 1], LOOK)
                if j > 0:
                    emit_C(j - 1)
            emit_normalize()
            emit_C(NTQ - 1, final=True)


def build_program():
    nc = bacc.Bacc(
        "TRN2", target_bir_lowering=False, debug=False, num_devices=NCORES
    )
    xT = nc.dram_tensor("xT", [D, T], BF16, kind="ExternalInput").ap()
    xg = nc.dram_tensor("xg", [T, GF], BF16, kind="ExternalInput").ap()
    wqT = nc.dram_tensor("wqT", [D, GF], BF16, kind="ExternalInput").ap()
    wkT = nc.dram_tensor("wkT", [D, GF], BF16, kind="ExternalInput").ap()
    woT = nc.dram_tensor("woT", [GF, D], BF16, kind="ExternalInput").ap()
    cT = nc.dram_tensor("cT", [HD, GH], F32, kind="ExternalInput").ap()
    bqT = nc.dram_tensor("bqT", [HD, GH], F32, kind="ExternalInput").ap()
    bkT = nc.dram_tensor("bkT", [HD, GH], F32, kind="ExternalInput").ap()
    ones = nc.dram_tensor("ones", [P, 1], BF16, kind="ExternalInput").ap()
    ident = nc.dram_tensor("ident", [P, P], BF16, kind="ExternalInput").ap()
    mtri = nc.dram_tensor("mtri", [P, P], BF16, kind="ExternalInput").ap()
    out = nc.dram_tensor("out", [D, T], BF16, kind="ExternalOutput").ap()

    with tile.TileContext(nc) as tc:
        _body(tc, xT, xg, wqT, wkT, woT, cT, bqT, bkT, ones, ident, mtri, out)
    nc.compile()
    return nc


_NC_CACHE = None
LAST_RESULT = None
TRACE = False


def kernel(x, Wq, bq, Wk, bk, Wvq, bvq, v_keys, v_embed, Wo, bo):
    global _NC_CACHE, LAST_RESULT
    bf = ml_dtypes.bfloat16
    x = np.asarray(x, np.float32)
    Wq = np.asarray(Wq, np.float32)
    bq = np.asarray(bq, np.float32)
    Wk = np.asarray(Wk, np.float32)
    bk = np.asarray(bk, np.float32)
    v_embed = np.asarray(v_embed, np.float32)
    Wo = np.asarray(Wo, np.float32)
    bo = np.asarray(bo, np.float32)

    c = 2.0 * v_embed[:G].sum(axis=0)
    p = np.arange(P)[:, None]
    f = np.arange(P)[None, :]
    mtri = np.where(f >= p, 0.0, NEG).astype(bf)
    in_maps = []
    for core in range(NCORES):
        b, g = divmod(core, G)
        gsl = slice(g * GF, (g + 1) * GF)
        in_maps.append(
            {
                "xT": np.ascontiguousarray(x[b].T.astype(bf)),
                "xg": np.ascontiguousarray(x[b][:, gsl].astype(bf)),
                "wqT": np.ascontiguousarray(Wq[gsl, :].T.astype(bf)),
                "wkT": np.ascontiguousarray(Wk[gsl, :].T.astype(bf)),
                "woT": np.ascontiguousarray(Wo[:, gsl].T.astype(bf)),
                "cT": np.ascontiguousarray(c[gsl].reshape(GH, HD).T),
                "bqT": np.ascontiguousarray(bq[gsl].reshape(GH, HD).T),
                "bkT": np.ascontiguousarray(bk[gsl].reshape(GH, HD).T),
                "ones": np.ones((P, 1), bf),
                "ident": np.eye(P, dtype=bf),
                "mtri": np.ascontiguousarray(mtri),
            }
        )

    if _NC_CACHE is None:
        _NC_CACHE = build_program()
    res = run_bass_kernel_spmd(
        _NC_CACHE, in_maps, list(range(NCORES)), trace=TRACE
    )
    LAST_RESULT = res

    out = np.zeros((B, T, D), np.float32)
    for core in range(NCORES):
        b = core // G
        out[b] += res.results[core]["out"].astype(np.float32).T
    out += bo[None, None, :]
    return out


if __name__ == "__main__":
    nc = build_program()
    print("built ok")
